# revision 1
# baseline (speedup 1.0000x reference)
"""DeepSeekV3-style MoE layer on 8 Trainium2 NeuronCores.

Sharding strategy (expert-parallel, host-orchestrated dispatch):
  - The router (tiny: T x H x E matmul + sigmoid + top-2, ~0.1% of FLOPs) is
    computed on host with jax-on-CPU, replicating the reference bit-exactly so
    routing decisions / tie-breaks match.
  - Core e receives the tokens routed to expert e (gathered + transposed +
    zero-padded to a shared capacity C), and expert e's weights with the
    per-expert scalar mean routing weight folded into the down projection.
  - The shared expert is data-parallel: core c processes tokens
    [c*256, (c+1)*256).
  - Host combine: scatter-add routed outputs, add shared outputs.

Device kernel: SwiGLU MLP with tokens on the matmul moving (free) dim and
hidden/intermediate dims on partitions.  Matmuls run as float32r (full-rate
fp32 mode on the PE, 1 cycle/row for free dim >= 256) giving ~2.5e-4 rel err
vs the fp32 reference; MOE_DTYPE=bf16/hybrid switch to bf16-class compute.
Expert/x tensors are cached in SBUF; the (large, per-core-identical) shared
expert weights stream through a small pool as m-chunk pairs, one contiguous
DMA each.  PSUM: 6 accumulation banks + 2 down-proj banks.
"""

import os

os.environ.setdefault("JAX_PLATFORMS", "axon,cpu")

import numpy as np

# Problem constants (hardcoded per spec nn_DeepSeekV3MoE_11269994184873).
H = 1024       # hidden size
I = 512        # moe intermediate size
E = 8          # routed experts == n cores
K = 2          # experts per token
SI = 1024      # shared expert intermediate
B, S = 2, 1024
T = B * S      # 2048 tokens
P = 128
N_CORES = 8
TS = T // 4        # shared-expert tokens per core (512): 4-way token split
SIH = SI // 2      # shared-expert intermediate half per core: 2-way SI split

_nc_cache: dict = {}
last_nc = None  # exposed for test harness (TimelineSim)


def _round_up(v, m):
    return ((v + m - 1) // m) * m


def _host_router(x, gate_w, lb_bias):
    """Replicate the reference router on CPU via jax (bit-exact scores/top-k)."""
    import jax
    import jax.numpy as jnp

    cpu = jax.devices("cpu")[0]
    with jax.default_device(cpu):
        xf = jnp.asarray(np.asarray(x, np.float32)).reshape(-1, H)
        logits = xf @ jnp.asarray(np.asarray(gate_w, np.float32)).T + jnp.asarray(
            np.asarray(lb_bias, np.float32)
        )
        scores = jax.nn.sigmoid(logits.astype(jnp.float32))
        topw, topi = jax.lax.top_k(scores, K)
        topw = (topw / (topw.sum(-1, keepdims=True) + 1e-8)).astype(jnp.float32)
        wmeans = []
        for e in range(E):
            m = topi == e
            cnt = m.sum()
            wmean = (topw * m).sum() / jnp.maximum(cnt, 1).astype(topw.dtype)
            wmeans.append(wmean)
        topi_np = np.asarray(topi)
        wmean_np = np.asarray(jnp.stack(wmeans), np.float32)
    return topi_np, wmean_np


def _build_bass(C, mode="f32r"):
    """Build the SPMD Bass program for capacity C (multiple of 64, >=256)."""
    from contextlib import ExitStack

    import concourse.bacc as bacc
    import concourse.mybir as mybir
    import concourse.tile as tile

    f32 = mybir.dt.float32
    f32r = mybir.dt.float32r
    bf16 = mybir.dt.bfloat16
    # DTI: dtype of gate/up operands (x, wg, wu, sg, su)
    # DTH: dtype of h and down-proj weights (wd, sd)
    DTI, DTH = {
        "f32r": (f32r, f32r),
        "bf16": (bf16, bf16),
        "hybrid": (bf16, f32r),
    }[mode]
    Silu = mybir.ActivationFunctionType.Silu

    nc = bacc.Bacc("TRN2", target_bir_lowering=False, debug=False,
                   num_devices=N_CORES)

    # DRAM I/O (per-core values, same shapes on every core)
    xe = nc.dram_tensor("xe", [H // P, P, C], DTI, kind="ExternalInput")
    wg = nc.dram_tensor("wg", [H // P, P, I], DTI, kind="ExternalInput")
    wu = nc.dram_tensor("wu", [H // P, P, I], DTI, kind="ExternalInput")
    wd = nc.dram_tensor("wd", [I // P, P, H], DTH, kind="ExternalInput")
    xs = nc.dram_tensor("xs", [H // P, P, TS], DTI, kind="ExternalInput")
    # shared weights streamed as m-chunk PAIRS: [m2, p, j, k, c] so each pair
    # is one contiguous DMA; each core holds only its SI-half slice
    sg = nc.dram_tensor("sg", [SIH // (2 * P), P, 2, H // P, P], DTI,
                        kind="ExternalInput")
    su = nc.dram_tensor("su", [SIH // (2 * P), P, 2, H // P, P], DTI,
                        kind="ExternalInput")
    sd = nc.dram_tensor("sd", [H // (2 * P), P, 2, SIH // P, P], DTH,
                        kind="ExternalInput")
    ye = nc.dram_tensor("ye", [H // P, P, C], f32, kind="ExternalOutput")
    zs = nc.dram_tensor("zs", [H // P, P, TS], f32, kind="ExternalOutput")

    KH = H // P    # 8 k-chunks for H contraction
    KI = I // P    # 4 k-chunks for I contraction
    KS = SIH // P  # 4 k-chunks for SI-half contraction

    # token tiles for the routed phase: balanced sizes, multiples of 64,
    # each >= 256 (fp32r needs free dim >= 256 for full rate)
    nt = max(1, -(-C // 512))
    units = C // 64
    a_tiles = []
    off = 0
    for i in range(nt):
        u = units // nt + (1 if i < units % nt else 0)
        a_tiles.append((off, u * 64))
        off += u * 64
    assert off == C and all(tn >= 256 or C < 256 for _, tn in a_tiles)
    max_tn = max(tn for _, tn in a_tiles)

    with tile.TileContext(nc) as tc:
        with ExitStack() as ctx:
            const = ctx.enter_context(tc.tile_pool(name="const", bufs=1))
            spool = ctx.enter_context(tc.tile_pool(name="stream", bufs=3))
            hpool = ctx.enter_context(tc.tile_pool(name="h", bufs=2))
            tpool = ctx.enter_context(tc.tile_pool(name="tmp", bufs=2))
            opool = ctx.enter_context(tc.tile_pool(name="out", bufs=3))
            # PSUM budget: 8 banks total = 6 "acc" + 2 "y" (shared across phases)
            psACC = ctx.enter_context(tc.tile_pool(name="psACC", bufs=5, space="PSUM"))
            psY = ctx.enter_context(tc.tile_pool(name="psY", bufs=3, space="PSUM"))

            # ---- static loads (per-k, interleaved so PE starts early) ----
            x_sb = const.tile([P, KH, C], DTI, tag="x_sb")
            wg_sb = const.tile([P, KH, I], DTI, tag="wg_sb")
            wu_sb = const.tile([P, KH, I], DTI, tag="wu_sb")
            for k in range(KH):
                nc.sync.dma_start(x_sb[:, k, :], xe[k])
                nc.sync.dma_start(wg_sb[:, k, :], wg[k])
                nc.sync.dma_start(wu_sb[:, k, :], wu[k])
            wd_sb = const.tile([P, KI, H], DTH, tag="wd_sb")
            for k in range(KI):
                nc.sync.dma_start(wd_sb[:, k, :], wd[k])
            xs_sb = const.tile([P, KH, TS], DTI, tag="xs_sb")
            nc.sync.dma_start(xs_sb[:], xs.ap().rearrange("k p t -> p k t"))

            # ---- interleaved emission: phase A token-tiles alternate with
            # phase B gate/up pairs so the (static) PE stream matches the DMA
            # arrival order ----
            h_s = const.tile([P, KS, TS], DTH, tag="h_s")
            npairs = SIH // (2 * P)

            def emit_phA_tile(off, tn):
                h_a = hpool.tile([P, KI, max_tn], DTH, tag="h_a", name=f"h_a{off}")
                xr = x_sb[:, :, off:off + tn]
                for m in range(I // P):
                    pg = psACC.tile([P, 512], f32, tag="acc", name=f"apg{off}_{m}")
                    pu = psACC.tile([P, 512], f32, tag="acc", name=f"apu{off}_{m}")
                    for k in range(KH):
                        nc.tensor.matmul(
                            pg[:, :tn],
                            wg_sb[:, k, m * P:(m + 1) * P],
                            xr[:, k, :],
                            start=(k == 0), stop=(k == KH - 1),
                        )
                    for k in range(KH):
                        nc.tensor.matmul(
                            pu[:, :tn],
                            wu_sb[:, k, m * P:(m + 1) * P],
                            xr[:, k, :],
                            start=(k == 0), stop=(k == KH - 1),
                        )
                    tg = tpool.tile([P, 512], f32, tag="tmp_silu",
                                    name=f"atg{off}_{m}")
                    nc.scalar.activation(tg[:, :tn], pg[:, :tn], Silu)
                    nc.vector.tensor_mul(h_a[:, m, :tn], tg[:, :tn], pu[:, :tn])
                y_sb = opool.tile([P, H // P, max_tn], f32, tag="y_sb",
                                  name=f"y_sb{off}")
                for m in range(H // P):
                    py = psY.tile([P, 512], f32, tag="y", name=f"apy{off}_{m}")
                    for k in range(KI):
                        nc.tensor.matmul(
                            py[:, :tn],
                            wd_sb[:, k, m * P:(m + 1) * P],
                            h_a[:, k, :tn],
                            start=(k == 0), stop=(k == KI - 1),
                        )
                    nc.any.tensor_copy(y_sb[:, m, :tn], py[:, :tn])
                nc.sync.dma_start(
                    ye.ap().rearrange("m p c -> p m c")[:, :, off:off + tn],
                    y_sb[:, :, :tn])

            _pf = {}

            def prefetch_phB_pair(m2):
                sgm = spool.tile([P, 2, KH, P], DTI, tag="sgm", name=f"sgm{m2}")
                nc.sync.dma_start(sgm[:], sg[m2])
                sum_ = spool.tile([P, 2, KH, P], DTI, tag="sum_", name=f"sum{m2}")
                nc.sync.dma_start(sum_[:], su[m2])
                _pf[m2] = (sgm, sum_)

            def emit_phB_pair(m2):
                if m2 in _pf:
                    sgm, sum_ = _pf.pop(m2)
                else:
                    sgm = spool.tile([P, 2, KH, P], DTI, tag="sgm",
                                     name=f"sgm{m2}")
                    nc.sync.dma_start(sgm[:], sg[m2])
                    sum_ = spool.tile([P, 2, KH, P], DTI, tag="sum_",
                                      name=f"sum{m2}")
                    nc.sync.dma_start(sum_[:], su[m2])
                for j in range(2):
                    m = 2 * m2 + j
                    pg = psACC.tile([P, 512], f32, tag="acc", name=f"bpg{m2}_{j}")
                    pu = psACC.tile([P, 512], f32, tag="acc", name=f"bpu{m2}_{j}")
                    for k in range(KH):
                        nc.tensor.matmul(
                            pg[:, :TS], sgm[:, j, k, :], xs_sb[:, k, :],
                            start=(k == 0), stop=(k == KH - 1),
                        )
                    for k in range(KH):
                        nc.tensor.matmul(
                            pu[:, :TS], sum_[:, j, k, :], xs_sb[:, k, :],
                            start=(k == 0), stop=(k == KH - 1),
                        )
                    ts_ = tpool.tile([P, 512], f32, tag="tmp_silu",
                                     name=f"bts{m2}_{j}")
                    nc.scalar.activation(ts_[:, :TS], pg[:, :TS], Silu)
                    nc.vector.tensor_mul(h_s[:, m, :], ts_[:, :TS], pu[:, :TS])

            for i in range(npairs):
                prefetch_phB_pair(i)
            _pfd = {}
            for i in range(len(a_tiles)):
                emit_phA_tile(*a_tiles[i])
            for i in range(npairs):
                emit_phB_pair(i)

            zre = zs.ap().rearrange("m p t -> p m t")
            for m2 in range(H // (2 * P)):
                if m2 in _pfd:
                    sdm = _pfd.pop(m2)
                else:
                    sdm = spool.tile([P, 2, KS, P], DTH, tag="sdm",
                                     name=f"sdm{m2}")
                    nc.sync.dma_start(sdm[:], sd[m2])
                z_sb = opool.tile([P, 2, TS], f32, tag="z_sb", name=f"z_sb{m2}")
                for j in range(2):
                    py = psY.tile([P, 512], f32, tag="y", name=f"bpy{m2}_{j}")
                    for k in range(KS):
                        nc.tensor.matmul(
                            py[:, :TS], sdm[:, j, k, :], h_s[:, k, :],
                            start=(k == 0), stop=(k == KS - 1),
                        )
                    nc.any.tensor_copy(z_sb[:, j, :], py[:, :TS])
                nc.sync.dma_start(zre[:, 2 * m2:2 * m2 + 2, :], z_sb[:])

    nc.finalize()
    return nc


DTYPE_MODE = os.environ.get("MOE_DTYPE", "f32r")


def _get_nc(C):
    global last_nc
    key = (C, DTYPE_MODE)
    if key not in _nc_cache:
        _nc_cache[key] = _build_bass(C, DTYPE_MODE)
    last_nc = _nc_cache[key]
    return _nc_cache[key]


def kernel(x, gate_w, lb_bias, expert_gate_w, expert_up_w, expert_down_w,
           shared_gate_w, shared_up_w, shared_down_w):
    from concourse.bass_utils import run_bass_kernel_spmd

    x = np.asarray(x, np.float32)
    gate_w = np.asarray(gate_w, np.float32)
    lb_bias = np.asarray(lb_bias, np.float32)
    egw = np.asarray(expert_gate_w, np.float32)
    euw = np.asarray(expert_up_w, np.float32)
    edw = np.asarray(expert_down_w, np.float32)
    sgw = np.asarray(shared_gate_w, np.float32)
    suw = np.asarray(shared_up_w, np.float32)
    sdw = np.asarray(shared_down_w, np.float32)

    xf = x.reshape(T, H)

    # ---- host router (replicates reference) ----
    topi, wmean = _host_router(x, gate_w, lb_bias)

    sel = [np.nonzero((topi == e).any(axis=-1))[0] for e in range(E)]
    counts = [len(s) for s in sel]
    C = max(_round_up(max(counts), 64), 256)

    nc = _get_nc(C)

    # ---- per-core inputs ----
    xfT = np.ascontiguousarray(xf.T)  # [H, T]

    # m-chunk-pair-major shared weights: [m2, p, j, k, c] with
    # lhsT[k*P+p, (2*m2+j)*P+c]
    def _pairs(wT, MD):
        # wT: [K_dim, MD] (already transposed weight)
        KD = wT.shape[0]
        a = wT.reshape(KD // P, P, MD // (2 * P), 2, P)   # [k, p, m2, j, c]
        return a.transpose(2, 1, 3, 0, 4)                 # [m2, p, j, k, c]

    # per-SI-half shared weights (core // 4 picks the half)
    sgT_h = [_pairs(sgw[h * SIH:(h + 1) * SIH].T, SIH) for h in range(2)]
    suT_h = [_pairs(suw[h * SIH:(h + 1) * SIH].T, SIH) for h in range(2)]
    sdT_h = [_pairs(np.ascontiguousarray(sdw[:, h * SIH:(h + 1) * SIH]).T, H)
             for h in range(2)]

    import ml_dtypes
    bfc = lambda a: np.ascontiguousarray(a).astype(ml_dtypes.bfloat16)
    f32c = lambda a: np.ascontiguousarray(a, np.float32)
    # cast_i: gate/up operands; cast_h: down-proj weights
    cast_i, cast_h = {
        "f32r": (f32c, f32c),
        "bf16": (bfc, bfc),
        "hybrid": (bfc, f32c),
    }[DTYPE_MODE]
    sgT_h = [cast_i(a) for a in sgT_h]
    suT_h = [cast_i(a) for a in suT_h]
    sdT_h = [cast_h(a) for a in sdT_h]
    in_maps = []
    for e in range(E):
        xe = np.zeros((H // P, P, C), np.float32)
        if counts[e]:
            xe.reshape(H, C)[:, :counts[e]] = xfT[:, sel[e]]
        wgT = cast_i(egw[e].T).reshape(H // P, P, I)
        wuT = cast_i(euw[e].T).reshape(H // P, P, I)
        wdT = cast_h((edw[e] * wmean[e]).T).reshape(I // P, P, H)
        tsl = e % 4    # token-slice index
        sh = e // 4    # SI half
        xs = cast_i(xfT[:, tsl * TS:(tsl + 1) * TS]).reshape(H // P, P, TS)
        in_maps.append({
            "xe": cast_i(xe), "wg": wgT, "wu": wuT, "wd": wdT,
            "xs": xs, "sg": sgT_h[sh], "su": suT_h[sh], "sd": sdT_h[sh],
        })

    res = run_bass_kernel_spmd(nc, in_maps, core_ids=list(range(N_CORES)))

    # ---- host combine ----
    out = np.zeros((T, H), np.float32)
    for e in range(E):
        if counts[e]:
            ye = res.results[e]["ye"].reshape(H, C)
            out[sel[e]] += ye[:, :counts[e]].T
        zsout = res.results[e]["zs"].reshape(H, TS)
        tsl = e % 4
        out[tsl * TS:(tsl + 1) * TS] += zsout.T
    return out.reshape(B, S, H).astype(x.dtype)



# revision 3
# speedup vs baseline: 1.1793x; 1.1793x over previous
"""DeepSeekV3-style MoE layer on 8 Trainium2 NeuronCores.

Sharding (expert-parallel, host-orchestrated dispatch):
  - Router (tiny) on host via jax-CPU, bit-exact with the reference.
  - Core e computes expert e over its routed tokens (gathered, transposed,
    zero-padded to capacity C) with wmean folded into the down projection.
  - Shared expert: 4-way token split x 2-way intermediate split.
  - Host combine: scatter-add routed outputs + shared outputs.

Device kernel (all matmul inputs fp16, f32 PSUM, fp16 staged outputs):
  Phase A (routed): token tiles [512, C-512].  The first tile runs the
  gate/up contraction k-outer across 8 parallel PSUM banks so the PE
  consumes k-chunks in DMA arrival order (minimal startup stall); the
  last two k-layers are m-staggered so silu/mul pipelining starts early.
  Phase B (shared): m-outer gate/up pairs.  Two phase-A down chunks and
  the tail-tile downs are deferred until after phase-B gate/up to cover
  the silu/mul latency ahead of phase-B downs.
"""

import os

os.environ.setdefault("JAX_PLATFORMS", "axon,cpu")

import numpy as np

# Problem constants (hardcoded per spec nn_DeepSeekV3MoE_11269994184873).
H = 1024       # hidden size
I = 512        # moe intermediate size
E = 8          # routed experts == n cores
K = 2          # experts per token
SI = 1024      # shared expert intermediate
B, S = 2, 1024
T = B * S      # 2048 tokens
P = 128
N_CORES = 8
TS = T // 4        # shared-expert tokens per core (512): 4-way token split
SIH = SI // 2      # shared-expert intermediate half per core: 2-way SI split
KH = H // P        # 8 k-chunks for H contraction
KI = I // P        # 4 k-chunks for I contraction
KS = SIH // P      # 4 k-chunks for SI-half contraction
MI = I // P        # 4 m-chunks of the routed intermediate
MH = H // P        # 8 m-chunks of hidden
MS = SIH // P      # 4 m-chunks of the shared intermediate half

_nc_cache: dict = {}
last_nc = None  # exposed for test harness (TimelineSim)


def _round_up(v, m):
    return ((v + m - 1) // m) * m


def _host_router(x, gate_w, lb_bias):
    """Replicate the reference router on CPU via jax (bit-exact scores/top-k)."""
    import jax
    import jax.numpy as jnp

    cpu = jax.devices("cpu")[0]
    with jax.default_device(cpu):
        xf = jnp.asarray(np.asarray(x, np.float32)).reshape(-1, H)
        logits = xf @ jnp.asarray(np.asarray(gate_w, np.float32)).T + jnp.asarray(
            np.asarray(lb_bias, np.float32)
        )
        scores = jax.nn.sigmoid(logits.astype(jnp.float32))
        topw, topi = jax.lax.top_k(scores, K)
        topw = (topw / (topw.sum(-1, keepdims=True) + 1e-8)).astype(jnp.float32)
        wmeans = []
        for e in range(E):
            m = topi == e
            cnt = m.sum()
            wmean = (topw * m).sum() / jnp.maximum(cnt, 1).astype(topw.dtype)
            wmeans.append(wmean)
        topi_np = np.asarray(topi)
        wmean_np = np.asarray(jnp.stack(wmeans), np.float32)
    return topi_np, wmean_np


def _build_bass(C):
    """Build the SPMD Bass program for capacity C (multiple of 16)."""
    from contextlib import ExitStack

    import concourse.bacc as bacc
    import concourse.mybir as mybir
    import concourse.tile as tile

    f32 = mybir.dt.float32
    f16 = mybir.dt.float16
    Silu = mybir.ActivationFunctionType.Silu

    nc = bacc.Bacc("TRN2", target_bir_lowering=False, debug=False,
                   num_devices=N_CORES)

    # DRAM I/O (per-core values, same shapes on every core), all fp16.
    xe = nc.dram_tensor("xe", [KH, P, C], f16, kind="ExternalInput")
    wg = nc.dram_tensor("wg", [KH, P, I], f16, kind="ExternalInput")
    wu = nc.dram_tensor("wu", [KH, P, I], f16, kind="ExternalInput")
    wd = nc.dram_tensor("wd", [KI, P, H], f16, kind="ExternalInput")
    xs = nc.dram_tensor("xs", [KH, P, TS], f16, kind="ExternalInput")
    sg = nc.dram_tensor("sg", [KH, P, SIH], f16, kind="ExternalInput")
    su = nc.dram_tensor("su", [KH, P, SIH], f16, kind="ExternalInput")
    sd = nc.dram_tensor("sd", [KS, P, H], f16, kind="ExternalInput")
    ye = nc.dram_tensor("ye", [MH, P, C], f16, kind="ExternalOutput")
    zs = nc.dram_tensor("zs", [MH, P, TS], f16, kind="ExternalOutput")

    # token tiles for the routed phase
    if C <= 512:
        a_tiles = [(0, C)]
    else:
        a_tiles = [(0, 512), (512, C - 512)]
    tn0 = a_tiles[0][1]

    with tile.TileContext(nc) as tc:
        with ExitStack() as ctx:
            const = ctx.enter_context(tc.tile_pool(name="const", bufs=1))
            tpool = ctx.enter_context(tc.tile_pool(name="tmp", bufs=2))
            psA = ctx.enter_context(tc.tile_pool(name="psA", bufs=4, space="PSUM"))
            psB = ctx.enter_context(tc.tile_pool(name="psB", bufs=4, space="PSUM"))

            # ---- SBUF tiles ----
            x_sb = const.tile([P, KH, C], f16, tag="x_sb")
            wg_sb = const.tile([P, KH, I], f16, tag="wg_sb")
            wu_sb = const.tile([P, KH, I], f16, tag="wu_sb")
            wd_sb = const.tile([P, KI, H], f16, tag="wd_sb")
            xs_sb = const.tile([P, KH, TS], f16, tag="xs_sb")
            sg_sb = const.tile([P, KH, SIH], f16, tag="sg_sb")
            su_sb = const.tile([P, KH, SIH], f16, tag="su_sb")
            sd_sb = const.tile([P, KS, H], f16, tag="sd_sb")
            h_a = const.tile([P, KI, C], f16, tag="h_a")
            h_s = const.tile([P, KS, TS], f16, tag="h_s")
            y_st = const.tile([P, MH, C], f16, tag="y_st")
            z_st = const.tile([P, MH, TS], f16, tag="z_st")

            # ---- input DMAs (SP queue, in arrival-order for the PE) ----
            def load_k(dst_sb, src, k0, k1):
                nc.sync.dma_start(
                    dst_sb[:, k0:k1, :],
                    src.ap()[k0:k1].rearrange("k p c -> p k c"),
                )

            for k0, k1 in [(0, 1), (1, 2), (2, 4), (4, 6), (6, 8)]:
                load_k(x_sb, xe, k0, k1)
                load_k(wg_sb, wg, k0, k1)
                load_k(wu_sb, wu, k0, k1)
            for k0, k1 in [(0, 2), (2, 4)]:
                load_k(wd_sb, wd, k0, k1)
            for k0, k1 in [(0, 4), (4, 8)]:
                load_k(xs_sb, xs, k0, k1)
            for k0, k1 in [(0, 4), (4, 8)]:
                load_k(sg_sb, sg, k0, k1)
            for k0, k1 in [(0, 4), (4, 8)]:
                load_k(su_sb, su, k0, k1)
            for k0, k1 in [(0, 2), (2, 4)]:
                load_k(sd_sb, sd, k0, k1)

            # ---- helpers ----
            _copy_flip = [0]

            def psum_copy(dst_ap, src_ap):
                # alternate Act / DVE for PSUM->fp16 staging copies
                if _copy_flip[0] & 1:
                    nc.scalar.copy(dst_ap, src_ap)
                else:
                    nc.vector.tensor_copy(dst_ap, src_ap)
                _copy_flip[0] += 1

            def silu_mul(pg, pu, h_tile, m, off, tn, name):
                tg = tpool.tile([P, 512], f32, tag="tg", name=f"tg{name}")
                nc.scalar.activation(tg[:, :tn], pg[:, :tn], Silu)
                nc.vector.tensor_mul(h_tile[:, m, off:off + tn], tg[:, :tn],
                                     pu[:, :tn])

            def gu_pair(w_g, w_u, x_t, h_tile, m, off, tn, nk, name):
                """m-outer gate/up pair for one m-chunk."""
                pg = psA.tile([P, 512], f32, tag="ps", name=f"pg{name}")
                for k in range(nk):
                    nc.tensor.matmul(pg[:, :tn], w_g[:, k, m * P:(m + 1) * P],
                                     x_t[:, k, off:off + tn],
                                     start=(k == 0), stop=(k == nk - 1))
                pu = psA.tile([P, 512], f32, tag="ps", name=f"pu{name}")
                for k in range(nk):
                    nc.tensor.matmul(pu[:, :tn], w_u[:, k, m * P:(m + 1) * P],
                                     x_t[:, k, off:off + tn],
                                     start=(k == 0), stop=(k == nk - 1))
                silu_mul(pg, pu, h_tile, m, off, tn, name)

            def down_chunk(w_d, h_tile, st_tile, m, off, tn, nk, name):
                py = psB.tile([P, 512], f32, tag="ps", name=f"py{name}")
                for k in range(nk):
                    nc.tensor.matmul(py[:, :tn], w_d[:, k, m * P:(m + 1) * P],
                                     h_tile[:, k, off:off + tn],
                                     start=(k == 0), stop=(k == nk - 1))
                psum_copy(st_tile[:, m, off:off + tn], py[:, :tn])

            # ---- phase A tile0 gate/up: k-outer startup ----
            # kA layers 0..5 across 8 parallel psums, then k6/k7 m-staggered.
            pgs = [psA.tile([P, 512], f32, tag="ps", name=f"pg0_{m}")
                   for m in range(MI)]
            pus = [psB.tile([P, 512], f32, tag="ps", name=f"pu0_{m}")
                   for m in range(MI)]
            for k in range(6):
                for m in range(MI):
                    nc.tensor.matmul(pgs[m][:, :tn0],
                                     wg_sb[:, k, m * P:(m + 1) * P],
                                     x_sb[:, k, 0:tn0],
                                     start=(k == 0), stop=False)
                    nc.tensor.matmul(pus[m][:, :tn0],
                                     wu_sb[:, k, m * P:(m + 1) * P],
                                     x_sb[:, k, 0:tn0],
                                     start=(k == 0), stop=False)
            for m in range(MI):
                for k in (6, 7):
                    nc.tensor.matmul(pgs[m][:, :tn0],
                                     wg_sb[:, k, m * P:(m + 1) * P],
                                     x_sb[:, k, 0:tn0],
                                     start=False, stop=(k == 7))
                for k in (6, 7):
                    nc.tensor.matmul(pus[m][:, :tn0],
                                     wu_sb[:, k, m * P:(m + 1) * P],
                                     x_sb[:, k, 0:tn0],
                                     start=False, stop=(k == 7))
                silu_mul(pgs[m], pus[m], h_a, m, 0, tn0, f"t0_{m}")

            # ---- phase A tail-tile gate/up (m-outer) ----
            for off, tn in a_tiles[1:]:
                for m in range(MI):
                    gu_pair(wg_sb, wu_sb, x_sb, h_a, m, off, tn, KH,
                            f"t1_{m}")

            # ---- phase A downs for tile0, m0..m5 ----
            for m in range(6):
                down_chunk(wd_sb, h_a, y_st, m, 0, tn0, KI, f"dt0_{m}")

            # ---- phase B gate/up (m-outer) ----
            for m in range(MS):
                gu_pair(sg_sb, su_sb, xs_sb, h_s, m, 0, TS, KH, f"b_{m}")

            # ---- deferred phase A downs: tile0 m6/m7 + tail tiles ----
            for m in (6, 7):
                down_chunk(wd_sb, h_a, y_st, m, 0, tn0, KI, f"dt0_{m}")
            for off, tn in a_tiles[1:]:
                for m in range(MH):
                    down_chunk(wd_sb, h_a, y_st, m, off, tn, KI, f"dt1_{m}")
            # routed output DMA (Act engine queue keeps SP free)
            nc.scalar.dma_start(
                ye.ap().rearrange("m p c -> p m c"), y_st[:, :, :])

            # ---- phase B downs ----
            for m in range(MH):
                down_chunk(sd_sb, h_s, z_st, m, 0, TS, KS, f"db_{m}")
                if m % 2 == 1:
                    nc.scalar.dma_start(
                        zs.ap()[m - 1:m + 1].rearrange("m p c -> p m c"),
                        z_st[:, m - 1:m + 1, :])

    nc.finalize()
    return nc


def _get_nc(C):
    global last_nc
    key = C
    if key not in _nc_cache:
        _nc_cache[key] = _build_bass(C)
    last_nc = _nc_cache[key]
    return _nc_cache[key]


def kernel(x, gate_w, lb_bias, expert_gate_w, expert_up_w, expert_down_w,
           shared_gate_w, shared_up_w, shared_down_w):
    from concourse.bass_utils import run_bass_kernel_spmd

    x = np.asarray(x, np.float32)
    gate_w = np.asarray(gate_w, np.float32)
    lb_bias = np.asarray(lb_bias, np.float32)
    egw = np.asarray(expert_gate_w, np.float32)
    euw = np.asarray(expert_up_w, np.float32)
    edw = np.asarray(expert_down_w, np.float32)
    sgw = np.asarray(shared_gate_w, np.float32)
    suw = np.asarray(shared_up_w, np.float32)
    sdw = np.asarray(shared_down_w, np.float32)

    xf = x.reshape(T, H)

    # ---- host router (replicates reference) ----
    topi, wmean = _host_router(x, gate_w, lb_bias)

    sel = [np.nonzero((topi == e).any(axis=-1))[0] for e in range(E)]
    counts = [len(s) for s in sel]
    C = max(_round_up(max(counts), 16), 128)

    nc = _get_nc(C)

    f16 = np.float16
    xfT = np.ascontiguousarray(xf.T)  # [H, T]
    xfT16 = xfT.astype(f16)

    # shared weights per SI-half (lhsT layouts)
    sgT_h = [np.ascontiguousarray(sgw[h * SIH:(h + 1) * SIH].T)
             .astype(f16).reshape(KH, P, SIH) for h in range(2)]
    suT_h = [np.ascontiguousarray(suw[h * SIH:(h + 1) * SIH].T)
             .astype(f16).reshape(KH, P, SIH) for h in range(2)]
    sdT_h = [np.ascontiguousarray(sdw[:, h * SIH:(h + 1) * SIH].T)
             .astype(f16).reshape(KS, P, H) for h in range(2)]

    in_maps = []
    for e in range(E):
        xe = np.zeros((H, C), f16)
        if counts[e]:
            xe[:, :counts[e]] = xfT16[:, sel[e]]
        wgT = np.ascontiguousarray(egw[e].T).astype(f16).reshape(KH, P, I)
        wuT = np.ascontiguousarray(euw[e].T).astype(f16).reshape(KH, P, I)
        wdT = np.ascontiguousarray((edw[e] * wmean[e]).T).astype(f16) \
            .reshape(KI, P, H)
        tsl = e % 4    # token-slice index
        sh = e // 4    # SI half
        xs = np.ascontiguousarray(
            xfT16[:, tsl * TS:(tsl + 1) * TS]).reshape(KH, P, TS)
        in_maps.append({
            "xe": xe.reshape(KH, P, C), "wg": wgT, "wu": wuT, "wd": wdT,
            "xs": xs, "sg": sgT_h[sh], "su": suT_h[sh], "sd": sdT_h[sh],
        })

    res = run_bass_kernel_spmd(nc, in_maps, core_ids=list(range(N_CORES)))

    # ---- host combine ----
    out = np.zeros((T, H), np.float32)
    for e in range(E):
        if counts[e]:
            ye = np.asarray(res.results[e]["ye"], np.float32).reshape(H, C)
            out[sel[e]] += ye[:, :counts[e]].T
        zsout = np.asarray(res.results[e]["zs"], np.float32).reshape(H, TS)
        tsl = e % 4
        out[tsl * TS:(tsl + 1) * TS] += zsout.T
    return out.reshape(B, S, H).astype(x.dtype)
